# revision 9
# baseline (speedup 1.0000x reference)
"""Trainium2 Bass kernel for nn_BivariateNormalAttention.

Self-contained: takes FULL inputs (B=16), shards batch across 8 NeuronCores
(2 images/core), runs a Bass/Tile kernel per core, gathers [16,8,56,56].

Pipeline per image:
  conv3x3(512->256)+BN+ReLU -> conv3x3(256->256)+BN+ReLU -> avgpool16 (7x7)
  -> conv3x3(256->128)+BN+ReLU -> conv3x3(128->128)+BN+ReLU -> avgpool3s2
  -> conv3x3(128->64)+BN+ReLU -> fc(576->128) -> bivariate-normal attention.

Convs 1-2 (99.9% of FLOPs) run as fp8-e4m3 DoubleRow matmuls (2 cin-chunks
contracted per instruction at 2x PE rate). x and the conv1 output stay
resident in SBUF (fp8), so the only bulk HBM traffic is the one-time x load.
Weights are scaled x32 before the fp8 cast; the scale is folded back in the
PSUM->SBUF activation. Convs 3-5 / fc / attention run in fp32.
"""
import sys
import numpy as np
import ml_dtypes

for _p in ("/opt/trn_rl_repo", "/root/.axon_site/_ro/trn_rl_repo"):
    if _p not in sys.path:
        sys.path.append(_p)

import concourse.bacc as bacc
import concourse.mybir as mybir
import concourse.tile as tile
from concourse.bass_utils import run_bass_kernel_spmd

F32 = mybir.dt.float32
FP8 = mybir.dt.float8e4
DR = mybir.MatmulPerfMode.DoubleRowSwInterleave

B, C, H, W = 16, 512, 112, 112
OUT, GMM = 8, 4
NCORE = 8
IMG = B // NCORE                 # 2 images per core
HP, WP = H + 2, W + 2            # 114
FLAT = HP * WP                   # 12996
C1LEN = 13008                    # FLAT padded to %16
RS = 4                           # conv strip rows
NSTRIP = H // RS                 # 28
NBLK = 7                         # 4-strip blocks
BROWS = 4 * RS + 2               # 18 rows per x block (incl halo)
XBLEN = 2064                     # BROWS*WP=2052 padded to %16
NFREE = RS * WP                  # 456
H2 = W2 = H // 2                 # 56
SIG2 = float(H) / 2.0            # sigma = 56
LOGR = float(np.log(3.0))
WSCALE = 32.0                    # fp8 weight pre-scale (power of 2)


def build_nc(r_loop=None, worder="ps", psa_bufs=6):
    """Bass program for one core processing IMG images."""
    nc = bacc.Bacc("TRN2", target_bir_lowering=False, debug=False)

    x = nc.dram_tensor("x", [IMG, 4, 128, FLAT], FP8, kind="ExternalInput")
    w1t = nc.dram_tensor("w1t", [128, 9, 2, 2, 256], FP8, kind="ExternalInput")
    w2t = nc.dram_tensor("w2t", [128, 9, 2, 2, 256], FP8, kind="ExternalInput")
    w3t = nc.dram_tensor("w3t", [128, 9, 2, 128], F32, kind="ExternalInput")
    w4t = nc.dram_tensor("w4t", [128, 9, 128], F32, kind="ExternalInput")
    w5t = nc.dram_tensor("w5t", [128, 9, 64], F32, kind="ExternalInput")
    wfct = nc.dram_tensor("wfct", [64, 9, 128], F32, kind="ExternalInput")
    b1d = nc.dram_tensor("b1d", [128, 2], F32, kind="ExternalInput")
    b2d = nc.dram_tensor("b2d", [128, 2], F32, kind="ExternalInput")
    b3d = nc.dram_tensor("b3d", [128, 1], F32, kind="ExternalInput")
    b4d = nc.dram_tensor("b4d", [128, 1], F32, kind="ExternalInput")
    b5d = nc.dram_tensor("b5d", [64, 1], F32, kind="ExternalInput")
    selpd = nc.dram_tensor("selpd", [128, 128], F32, kind="ExternalInput")
    selgd = nc.dram_tensor("selgd", [32, 8], F32, kind="ExternalInput")
    negiod = nc.dram_tensor("negiod", [32, 56], F32, kind="ExternalInput")
    cstd = nc.dram_tensor("cstd", [32, 1], F32, kind="ExternalInput")  # -ln(3)

    out = nc.dram_tensor("out", [IMG, OUT, H2, W2], F32, kind="ExternalOutput")
    paccd = nc.dram_tensor("paccd", [IMG, 2, 128, 49], F32,
                           kind="ExternalOutput")

    with tile.TileContext(nc) as tc:
        with (
            tc.tile_pool(name="persist", bufs=1) as pp,
            tc.tile_pool(name="et", bufs=3) as etp,
            tc.tile_pool(name="hc", bufs=1) as hc,
            tc.tile_pool(name="att", bufs=1) as attp,
            tc.tile_pool(name="psa", bufs=psa_bufs, space="PSUM") as psa,
            tc.tile_pool(name="psc", bufs=2, space="PSUM") as psc,
        ):
            # ---------------- persistent tiles ----------------
            xb = [pp.tile([128, 4, XBLEN], FP8, name=f"xb{b}", tag=f"xb{b}")
                  for b in range(NBLK)]
            c1 = pp.tile([128, 2, C1LEN], FP8, tag="c1")
            w1 = pp.tile([128, 9, 2, 2, 256], FP8, tag="w1")
            w2 = pp.tile([128, 9, 2, 2, 256], FP8, tag="w2")
            w3 = pp.tile([128, 9, 2, 128], F32, tag="w3")
            w4 = pp.tile([128, 9, 128], F32, tag="w4")
            w5 = pp.tile([128, 9, 64], F32, tag="w5")
            wfc = pp.tile([64, 9, 128], F32, tag="wfc")
            b1 = pp.tile([128, 2], F32, tag="b1")
            b2 = pp.tile([128, 2], F32, tag="b2")
            b3 = pp.tile([128, 1], F32, tag="b3")
            b4 = pp.tile([128, 1], F32, tag="b4")
            b5 = pp.tile([64, 1], F32, tag="b5")
            selp = pp.tile([128, 128], F32, tag="selp")
            selg = pp.tile([32, 8], F32, tag="selg")
            negio = pp.tile([32, 56], F32, tag="negio")
            cst = pp.tile([32, 1], F32, tag="cst")
            pacc = [[pp.tile([128, 49], F32, name=f"pacc{i}_{c}",
                             tag=f"pacc{i}_{c}")
                     for c in range(2)] for i in range(IMG)]

            # ---------------- prologue (outside r_loop) ----------------
            for tdst, tsrc in ((w1, w1t), (w2, w2t), (w3, w3t), (w4, w4t),
                               (w5, w5t), (wfc, wfct), (b1, b1d), (b2, b2d),
                               (b3, b3d), (b4, b4d), (b5, b5d), (selp, selpd),
                               (selg, selgd), (negio, negiod), (cst, cstd)):
                nc.gpsimd.dma_start(tdst[:], tsrc[:])
            # zero c1 once: interior rewritten every image, borders stay 0
            nc.gpsimd.memset(
                c1[:].rearrange("p c f -> p (c f)").bitcast(F32), 0.0)
            # zero x block pads [2052:2064) once (never written by DMA)
            for b in range(NBLK):
                nc.vector.memset(
                    xb[b][:].rearrange("p c f -> p (c f)").bitcast(F32)
                    .rearrange("p (c f) -> p c f", f=XBLEN // 4)[:, :, 513:516],
                    0.0)

            def load_x(img):
                for b in range(NBLK):
                    nc.gpsimd.dma_start(
                        xb[b][:, :, 0:BROWS * WP],
                        x[img].rearrange("c p f -> p c f")
                        [:, :, 16 * b * WP:16 * b * WP + BROWS * WP])

            def conv1(img):
                for blk in range(NBLK):
                    for co in range(2):
                        ps = [psa.tile([128, NFREE], F32, name="ps",
                                       tag="ps")
                              for _ in range(4)]
                        pairs = [(t, cp) for t in range(9) for cp in range(2)]
                        if worder == "ps":
                            seq = [(p, s) for p in range(18) for s in range(4)]
                        else:
                            seq = [(p, s) for s in range(4) for p in range(18)]
                        for p, s4 in seq:
                            t, cp = pairs[p]
                            base = 4 * s4 * WP + (t // 3) * WP + t % 3
                            nc.tensor.matmul(
                                ps[s4][:],
                                w1[:, t, cp, co, :]
                                .rearrange("p (a b) -> p a b", b=128),
                                xb[blk][:, 2 * cp:2 * cp + 2, base:base + NFREE],
                                start=(p == 0), stop=(p == 17), perf_mode=DR)
                        for s4 in range(4):
                            srow = 4 * (4 * blk + s4)
                            nc.scalar.activation(
                                c1[:, co, 0:FLAT]
                                .rearrange("p (r c) -> p r c", c=WP)
                                [:, 1 + srow:5 + srow, 1:113],
                                ps[s4][:].rearrange("p (r c) -> p r c", c=WP)
                                [:, :, 0:112],
                                mybir.ActivationFunctionType.Relu,
                                bias=b1[:, co:co + 1], scale=1.0 / WSCALE)

            def conv2(img):
                for c in range(2):
                    nc.vector.memset(pacc[img][c][:], 0.0)
                for blk in range(NBLK):
                    for co in range(2):
                        ps = [psa.tile([128, NFREE], F32, name="ps",
                                       tag="ps")
                              for _ in range(4)]
                        if worder == "ps":
                            seq = [(p, s) for p in range(18) for s in range(4)]
                        else:
                            seq = [(p, s) for s in range(4) for p in range(18)]
                        for p, s4 in seq:
                            t, hl = p // 2, p % 2
                            s = 4 * blk + s4
                            base = 4 * s * WP + (t // 3) * WP + t % 3
                            nc.tensor.matmul(
                                ps[s4][:],
                                w2[:, t, hl, co, :]
                                .rearrange("p (a b) -> p a b", b=128),
                                c1[:, 0:2, base:base + NFREE],
                                start=(p == 0), stop=(p == 17), perf_mode=DR)
                        for s4 in range(4):
                            et = etp.tile([128, RS, WP], mybir.dt.bfloat16,
                                          tag=f"et{co}")
                            nc.scalar.activation(
                                et[:],
                                ps[s4][:].rearrange("p (a b) -> p a b", b=WP),
                                mybir.ActivationFunctionType.Relu,
                                bias=b2[:, co:co + 1], scale=1.0 / WSCALE)
                            rs_ = etp.tile([128, 7], F32, tag=f"rs{co}")
                            nc.vector.reduce_sum(
                                rs_[:],
                                et[:, :, 0:112].rearrange(
                                    "p r (g c) -> p g r c", c=16),
                                axis=mybir.AxisListType.XY)
                            nc.vector.tensor_add(
                                pacc[img][co][:, blk * 7:(blk + 1) * 7],
                                pacc[img][co][:, blk * 7:(blk + 1) * 7],
                                rs_[:])

            def dump_pacc(img):
                for co in range(2):
                    nc.gpsimd.dma_start(paccd[img, co], pacc[img][co][:])

            def head(img):
                # conv3 (7x7, 256->128): padded 9x9 inputs (+2 tail)
                p3in = []
                for ci in range(2):
                    pi = hc.tile([128, 83], F32, name=f"p3in{ci}",
                                 tag=f"p3in{ci}")
                    nc.vector.memset(pi[:], 0.0)
                    nc.vector.tensor_copy(
                        pi[:, 10:73].rearrange("p (a b) -> p a b", b=9)[:, :, 0:7],
                        pacc[img][ci][:].rearrange("p (a b) -> p a b", b=7))
                    p3in.append(pi)
                ps3 = psc.tile([128, 448], F32, name="ps3", tag="psh")[:, 0:63]
                k = 0
                for ci in range(2):
                    for t in range(9):
                        nc.tensor.matmul(
                            ps3, w3[:, t, ci, :],
                            p3in[ci][:, (t // 3) * 9 + t % 3:
                                     (t // 3) * 9 + t % 3 + 63],
                            start=(k == 0), stop=(k == 17))
                        k += 1
                p4in = hc.tile([128, 83], F32, tag="p4in")
                nc.vector.memset(p4in[:], 0.0)
                nc.scalar.activation(
                    p4in[:, 10:73].rearrange("p (a b) -> p a b", b=9)[:, :, 0:7],
                    ps3.rearrange("p (a b) -> p a b", b=9)[:, :, 0:7],
                    mybir.ActivationFunctionType.Relu, bias=b3[:, 0:1])
                # conv4 (7x7, 128->128)
                ps4 = psc.tile([128, 448], F32, name="ps4", tag="psh")[:, 0:63]
                for t in range(9):
                    nc.tensor.matmul(
                        ps4, w4[:, t, :],
                        p4in[:, (t // 3) * 9 + t % 3:
                             (t // 3) * 9 + t % 3 + 63],
                        start=(t == 0), stop=(t == 8))
                c4t = hc.tile([128, 49], F32, tag="c4t")
                nc.scalar.activation(
                    c4t[:].rearrange("p (a b) -> p a b", b=7),
                    ps4.rearrange("p (a b) -> p a b", b=9)[:, :, 0:7],
                    mybir.ActivationFunctionType.Relu, bias=b4[:, 0:1])
                # avgpool 3x3 stride 2 (sum; /9 folded into w5)
                c4v = c4t[:].rearrange("p (y x) -> p y x", x=7)
                a1 = hc.tile([128, 7, 3], F32, tag="a1")
                nc.vector.tensor_add(a1[:], c4v[:, :, 0:5:2], c4v[:, :, 1:6:2])
                nc.vector.tensor_add(a1[:], a1[:], c4v[:, :, 2:7:2])
                a2 = hc.tile([128, 9], F32, tag="a2")
                a2v = a2[:].rearrange("p (i j) -> p i j", j=3)
                nc.vector.tensor_add(a2v, a1[:, 0:5:2, :], a1[:, 1:6:2, :])
                nc.vector.tensor_add(a2v, a2v, a1[:, 2:7:2, :])
                # conv5 (3x3, 128->64): padded 5x5 (+2 tail)
                p5in = hc.tile([128, 27], F32, tag="p5in")
                nc.vector.memset(p5in[:], 0.0)
                nc.vector.tensor_copy(
                    p5in[:, 6:21].rearrange("p (a b) -> p a b", b=5)[:, :, 0:3],
                    a2[:].rearrange("p (a b) -> p a b", b=3))
                ps5 = psc.tile([128, 448], F32, name="ps5", tag="psh")[0:64, 0:15]
                for t in range(9):
                    nc.tensor.matmul(
                        ps5, w5[:, t, :],
                        p5in[:, (t // 3) * 5 + t % 3:
                             (t // 3) * 5 + t % 3 + 15],
                        start=(t == 0), stop=(t == 8))
                h5 = hc.tile([64, 9], F32, tag="h5")
                nc.scalar.activation(
                    h5[:].rearrange("p (a b) -> p a b", b=3),
                    ps5.rearrange("p (a b) -> p a b", b=5)[:, :, 0:3],
                    mybir.ActivationFunctionType.Relu, bias=b5[:, 0:1])
                # fc 576->128 as 9 accumulating matmuls (K=64)
                psf = psc.tile([128, 448], F32, name="psf", tag="psh")[:, 0:1]
                for t in range(9):
                    nc.tensor.matmul(psf, wfc[:, t, :], h5[:, t:t + 1],
                                     start=(t == 0), stop=(t == 8))
                sig = hc.tile([128, 1], F32, tag="sig")
                nc.scalar.activation(sig[:], psf,
                                     mybir.ActivationFunctionType.Sigmoid)
                # params: one selector matmul -> [mx | my | t | rho']
                psl = psc.tile([128, 448], F32, name="psl", tag="psh")[:, 0:1]
                nc.tensor.matmul(psl, selp[:], sig[:], start=True, stop=True)
                mx = hc.tile([32, 1], F32, tag="mx")
                my = hc.tile([32, 1], F32, tag="my")
                nc.vector.tensor_copy(mx[:], psl[0:32])
                nc.vector.tensor_copy(my[:], psl[32:64])
                r32 = hc.tile([32, 1], F32, tag="r32")
                nc.scalar.activation(r32[:], psl[64:96],
                                     mybir.ActivationFunctionType.Exp,
                                     bias=cst[:, 0:1])
                rho = hc.tile([32, 1], F32, tag="rho")
                nc.vector.tensor_scalar(rho[:], psl[96:128], -0.8, None,
                                        mybir.AluOpType.add)
                rr = hc.tile([32, 1], F32, tag="rr")
                nc.vector.tensor_mul(rr[:], rho[:], rho[:])
                om = hc.tile([32, 1], F32, tag="om")
                nc.vector.tensor_scalar(om[:], rr[:], -1.0, 1.0,
                                        mybir.AluOpType.mult,
                                        mybir.AluOpType.add)
                iom = hc.tile([32, 1], F32, tag="iom")
                nc.vector.reciprocal(iom[:], om[:])
                den = hc.tile([32, 1], F32, tag="den")
                nc.vector.tensor_scalar(den[:], iom[:],
                                        -0.5 / (SIG2 * SIG2), None,
                                        mybir.AluOpType.mult)
                ai = hc.tile([32, 1], F32, tag="ai")
                nc.vector.tensor_mul(ai[:], den[:], r32[:])
                ir = hc.tile([32, 1], F32, tag="ir")
                nc.vector.reciprocal(ir[:], r32[:])
                bj = hc.tile([32, 1], F32, tag="bj")
                nc.vector.tensor_mul(bj[:], den[:], ir[:])
                cc = hc.tile([32, 1], F32, tag="cc")
                nc.vector.scalar_tensor_tensor(
                    cc[:], den[:], -2.0, rho[:],
                    mybir.AluOpType.mult, mybir.AluOpType.mult)
                dx = hc.tile([32, 56], F32, tag="dx")
                nc.vector.tensor_scalar(dx[:], negio[:], mx[:, 0:1], None,
                                        mybir.AluOpType.add)
                dy = hc.tile([32, 56], F32, tag="dy")
                nc.vector.tensor_scalar(dy[:], negio[:], my[:, 0:1], None,
                                        mybir.AluOpType.add)
                u = hc.tile([32, 56], F32, tag="u")
                nc.vector.scalar_tensor_tensor(
                    u[:], dx[:], ai[:, 0:1], dx[:],
                    mybir.AluOpType.mult, mybir.AluOpType.mult)
                v = hc.tile([32, 56], F32, tag="v")
                nc.vector.scalar_tensor_tensor(
                    v[:], dy[:], bj[:, 0:1], dy[:],
                    mybir.AluOpType.mult, mybir.AluOpType.mult)
                lt = attp.tile([32, 56, 56], F32, tag="lt")
                nc.vector.scalar_tensor_tensor(
                    lt[:], dx[:].unsqueeze(2).broadcast_to([32, 56, 56]),
                    cc[:, 0:1],
                    dy[:].unsqueeze(1).broadcast_to([32, 56, 56]),
                    mybir.AluOpType.mult, mybir.AluOpType.mult)
                nc.vector.tensor_add(
                    lt[:], lt[:],
                    u[:].unsqueeze(2).broadcast_to([32, 56, 56]))
                nc.vector.tensor_add(
                    lt[:], lt[:],
                    v[:].unsqueeze(1).broadcast_to([32, 56, 56]))
                att = attp.tile([32, 56 * 56], F32, tag="att")
                asum = hc.tile([32, 1], F32, tag="asum")
                nc.scalar.activation(
                    att[:], lt[:].rearrange("p a b -> p (a b)"),
                    mybir.ActivationFunctionType.Exp,
                    accum_out=asum[:])
                inv = hc.tile([32, 1], F32, tag="inv")
                nc.vector.reciprocal(inv[:], asum[:])
                nc.vector.tensor_scalar(att[:], att[:], inv[:, 0:1], None,
                                        mybir.AluOpType.mult)
                obuf = attp.tile([8, 56 * 56], F32, tag="obuf")
                for ch in range(7):
                    pso = psc.tile([128, 448], F32, name="pso", tag="psh")[0:8, :]
                    nc.tensor.matmul(pso, selg[:],
                                     att[:, ch * 448:(ch + 1) * 448],
                                     start=True, stop=True)
                    nc.vector.tensor_copy(
                        obuf[:, ch * 448:(ch + 1) * 448], pso)
                nc.gpsimd.dma_start(
                    out[img].rearrange("o a b -> o (a b)"), obuf[:])

            def emit_body():
                load_x(0)
                conv1(0)
                load_x(1)
                conv2(0)
                dump_pacc(0)
                conv1(1)
                head(0)
                conv2(1)
                dump_pacc(1)
                head(1)

            if r_loop:
                with tc.For_i(0, r_loop, 1):
                    emit_body()
            else:
                emit_body()
    nc.compile()
    return nc


def prep_inputs(inputs):
    """Host prep: fold BN/pool scales, build device layouts, shard batch."""
    x = inputs["x"]
    eps_s = 1.0 / np.sqrt(np.float32(1.0 + 1e-5))
    FP8NP = ml_dtypes.float8_e4m3

    def fold(w, g):
        s = (g * eps_s).astype(np.float32)
        return (w * s[:, None, None, None]).astype(np.float32)

    w1 = fold(inputs["w1"], inputs["g1"]) * WSCALE    # [256,512,3,3]
    w2 = fold(inputs["w2"], inputs["g2"]) * WSCALE    # [256,256,3,3]
    w3 = fold(inputs["w3"], inputs["g3"]) / 256.0     # avgpool16 norm
    w4 = fold(inputs["w4"], inputs["g4"])
    w5 = fold(inputs["w5"], inputs["g5"]) / 9.0       # avgpool3 norm
    wfc = np.asarray(inputs["w_fc"], np.float32)      # [128, 576]
    mw = np.asarray(inputs["mix_w"], np.float32).reshape(OUT, GMM)
    mw = np.exp(mw - mw.max(1, keepdims=True))
    mw = mw / mw.sum(1, keepdims=True)                # softmax over gmm

    # conv weights -> [128(p=cin%128), 9(tap), ncin, cout]
    def wt_layout(w, ncin):
        co = w.shape[0]
        r = w.transpose(1, 2, 3, 0).reshape(ncin, 128, 9, co)  # [ncin,128,9,co]
        return np.ascontiguousarray(r.transpose(1, 2, 0, 3))   # [128,9,ncin,co]

    FP8NP = ml_dtypes.float8_e4m3

    def swi_pack(a, b):
        # a, b: [128, 9, G, 2(coc), 128(m)] fp8 -> [128, 9, G, 2, 256] with
        # raw[..., 2m + i] = (a if i == 0 else b)[..., 127 - m]
        ar = a[..., ::-1]
        br = b[..., ::-1]
        st = np.stack([ar, br], axis=-1)              # [...,128,2]
        return np.ascontiguousarray(st.reshape(*st.shape[:-2], 256))

    w1f = wt_layout(w1, 4)                            # [128,9,4,256] f32
    w1v = w1f.reshape(128, 9, 2, 2, 2, 128)           # [p,t,cp,ab,coc,m]
    w1q = w1v.astype(FP8NP)
    w1t = swi_pack(w1q[:, :, :, 0].transpose(0, 1, 2, 3, 4),
                   w1q[:, :, :, 1].transpose(0, 1, 2, 3, 4))
    # -> [128,9,2(cp),2(coc),256]
    w2s = wt_layout(w2, 2)                            # [128,9,2(ci),256] f32
    w2hi = w2s.astype(FP8NP)
    w2lo = (w2s - w2hi.astype(np.float32)).astype(FP8NP)

    def w2_swi(part):                                 # part: [128,9,2,256] fp8
        v = part.reshape(128, 9, 2, 2, 128)           # [p,t,ci,coc,m]
        return swi_pack(v[:, :, 0][:, :, None], v[:, :, 1][:, :, None])[:, :, 0]

    w2t = np.ascontiguousarray(
        np.stack([w2_swi(w2hi), w2_swi(w2lo)], axis=2))  # [128,9,2(hl),2,256]
    w3t = wt_layout(w3, 2)
    w4t = wt_layout(w4, 1)[:, :, 0, :]
    w5t = wt_layout(w5, 1)[:, :, 0, :]
    wfct = np.ascontiguousarray(wfc.reshape(128, 64, 9).transpose(1, 2, 0))

    def bias_chunks(b, nchunk):
        return np.ascontiguousarray(
            np.asarray(b, np.float32).reshape(nchunk, 128).T)

    b1h = bias_chunks(inputs["b1"], 2)
    b2h = bias_chunks(inputs["b2"], 2)
    b3h = np.asarray(inputs["b3"], np.float32).reshape(128, 1)
    b4h = np.asarray(inputs["b4"], np.float32).reshape(128, 1)
    b5h = np.asarray(inputs["b5"], np.float32).reshape(64, 1)

    selp = np.zeros((128, 128), np.float32)
    for m in range(32):
        selp[4 * m + 0, m] = float(H2 - 1)
        selp[4 * m + 1, m + 32] = float(W2 - 1)
        selp[4 * m + 2, m + 64] = 2.0 * LOGR
        selp[4 * m + 3, m + 96] = 1.6
    selg = np.zeros((32, 8), np.float32)
    for o in range(OUT):
        for g in range(GMM):
            selg[o * GMM + g, o] = mw[o, g]
    negio = np.broadcast_to(-np.arange(56, dtype=np.float32), (32, 56)).copy()
    cst = np.full((32, 1), -LOGR, np.float32)

    xp = np.zeros((B, 4, 128, HP, WP), FP8NP)
    xp[:, :, :, 1:113, 1:113] = np.asarray(x, np.float32).reshape(
        B, 4, 128, H, W).astype(FP8NP)
    xp = xp.reshape(B, 4, 128, FLAT)

    common = {
        "w1t": w1t, "w2t": w2t,
        "w3t": w3t, "w4t": w4t, "w5t": w5t, "wfct": wfct,
        "b1d": b1h, "b2d": b2h, "b3d": b3h, "b4d": b4h, "b5d": b5h,
        "selpd": selp, "selgd": selg, "negiod": negio, "cstd": cst,
    }
    in_maps = []
    for c in range(NCORE):
        m = dict(common)
        m["x"] = np.ascontiguousarray(xp[c * IMG:(c + 1) * IMG])
        in_maps.append(m)
    return in_maps


_CACHE = {}


def kernel(**inputs):
    inputs = {k: np.asarray(v) for k, v in inputs.items()}
    if "nc" not in _CACHE:
        _CACHE["nc"] = build_nc()
    nc = _CACHE["nc"]
    in_maps = prep_inputs(inputs)
    res = run_bass_kernel_spmd(nc, in_maps, core_ids=list(range(NCORE)))
    out = np.concatenate([res.results[c]["out"] for c in range(NCORE)], axis=0)
    return np.ascontiguousarray(out.astype(np.float32))
